# revision 12
# baseline (speedup 1.0000x reference)
"""Tensor-parallel causal GQA self-attention (B=1, S=2048, D=4096, 32 q heads /
8 kv heads, HD=128, interleaved RoPE) on 8 trn2 NeuronCores.

Sharding: core c owns kv head c and q heads 4c..4c+3 (column-parallel
Wq/Wk/Wv, row-parallel Wo).  Each core computes a full [S, D] partial of the
output projection; the host sums the 8 partials (the "all-reduce").

All matmul operands are bf16 (same 1 col/cycle PE rate as fp32r, half the
DMA/SBUF footprint, and no small-N fp32r penalty); PSUM accumulation stays
fp32.  On-device layout:
  QT/KT  [HD, S]   transposed per-head layouts straight out of the
                   projection matmuls (lhsT = W d-tile, rhs = xT d-tile)
  RoPE   applied in [HD, S] layout: rope(q) = q*C2 + perm(q)*S2, where
                   perm is a 128x128 pair-swap matmul and C2/S2 host tables
  scores ST[j, i] = KT_tile.T @ QT_chunk; diagonal j-tiles only compute the
         live query columns [128m:512] (bf16 takes no small-N penalty), the
         dead columns are zeroed by a DVE memset and the 128-wide triangular
         boundary block is masked in place (exp(s)*binmask)
  P      exp(ST) via ScalarE straight from PSUM into a per-head contiguous
         [128, 512*njt] bf16 buffer
  denom  per-512-key-group cross-partition reduce on GpSimd into a f32
         [1, 512*njt] partials row, restriped to [njt, 512] by one
         SBUF->SBUF DMA, final GpSimd C-reduce + DVE reciprocal + GpSimd
         partition_broadcast -> [128,512] bf16 reciprocal tile.  Softmax
         normalization costs ZERO TensorE cycles.
  AV     OT[h, i] += V_tile.T(natural) @ PT_tile  (full width; masked P
         columns are exact zeros)
  o_proj out[s, dout] += OTn_tile.T @ Wo_tile, chunk-major so late
         attention tails are only needed by the last quarter

DMA-issue cost dominates naive streaming, so transfers are batched via
host-side interleaved bf16 layouts:
  xw   [4, 16, 128, 1536]  x-chunk + wkv for TWO d-tiles per DMA
  wq2  [16, 128, 1024]     two wq d-tiles per DMA (ACT hwdge ring)
  wo4  [8, 2, 128, 1024]   all four head-blocks of a Wo dout-chunk, 2 DMAs
  out  [8, 8, 128, 1024]   staged bf16 [128,1024] stores, host re-assembles
"""

import sys

if "/opt/trn_rl_repo" not in sys.path:
    sys.path.insert(0, "/opt/trn_rl_repo")

import numpy as np
import ml_dtypes

import concourse.bass as bass
import concourse.tile as tile
from concourse import bacc, library_config, mybir
from concourse.bass_utils import run_bass_kernel_spmd

S, D, NH, NKV, HD = 2048, 4096, 32, 8, 128
NCORES = 8
QH = NH // NCORES  # 4 q heads per core
ROPE_BASE = 500000.0

F32 = mybir.dt.float32
BF16 = mybir.dt.bfloat16
AF = mybir.ActivationFunctionType
BF = ml_dtypes.bfloat16

SC = S // 512   # 4 s-chunks of 512
DP = D // 256   # 16 d-tile pairs
JT = S // 128   # 16 j-tiles of 128

_CACHE = {}

# set by test harness to collect an exec-time profile
TRACE = False
LAST_EXEC_NS = None


def _build_nc():
    nc = bacc.Bacc("TRN2", target_bir_lowering=False, debug=False,
                   num_devices=NCORES)

    xw_d = nc.declare_dram_parameter("xw", [SC, DP, 128, 1536], BF16,
                                     isOutput=False)
    wq_d = nc.declare_dram_parameter("wq2", [DP, 128, 1024], BF16,
                                     isOutput=False)
    wo_d = nc.declare_dram_parameter("wo4", [8, 2, 128, 1024], BF16,
                                     isOutput=False)
    cos_d = nc.declare_dram_parameter("cos2", [SC, HD, 512], BF16,
                                      isOutput=False)
    sin_d = nc.declare_dram_parameter("sin2", [SC, HD, 512], BF16,
                                      isOutput=False)
    perm_d = nc.declare_dram_parameter("perm", [HD, HD], BF16, isOutput=False)
    tri_d = nc.declare_dram_parameter("tri", [128, 128], BF16, isOutput=False)
    ident_d = nc.declare_dram_parameter("ident", [HD, HD], BF16,
                                        isOutput=False)
    out_d = nc.declare_dram_parameter("out", [8, 8, 128, 1024], BF16,
                                      isOutput=True)

    with tile.TileContext(nc) as tc:
        from contextlib import ExitStack
        ctx = ExitStack()
        with ctx:
            wpool = ctx.enter_context(tc.tile_pool(name="wpool", bufs=16))
            xpool = ctx.enter_context(tc.tile_pool(name="xpool", bufs=3))
            qtp = ctx.enter_context(tc.tile_pool(name="qtp", bufs=9))
            otnp = ctx.enter_context(tc.tile_pool(name="otnp", bufs=16))
            otup = ctx.enter_context(tc.tile_pool(name="otup", bufs=2))
            tabp = ctx.enter_context(tc.tile_pool(name="tabp", bufs=4))
            ktp = ctx.enter_context(tc.tile_pool(name="ktp", bufs=4))
            vnp = ctx.enter_context(tc.tile_pool(name="vnp", bufs=4))
            stg = ctx.enter_context(tc.tile_pool(name="stg", bufs=4))
            rawp = ctx.enter_context(tc.tile_pool(name="rawp", bufs=6))
            ptp = ctx.enter_context(tc.tile_pool(name="ptp", bufs=4))
            denp = ctx.enter_context(tc.tile_pool(name="denp", bufs=2))
            p1p = ctx.enter_context(tc.tile_pool(name="p1p", bufs=1))
            cst = ctx.enter_context(tc.tile_pool(name="cst", bufs=1))
            ostp = ctx.enter_context(tc.tile_pool(name="ostp", bufs=3))
            psA = ctx.enter_context(
                tc.tile_pool(name="psA", bufs=6, space=bass.MemorySpace.PSUM))
            psB = ctx.enter_context(
                tc.tile_pool(name="psB", bufs=2, space=bass.MemorySpace.PSUM))

            nc.gpsimd.load_library(library_config.attn)

            perm_t = cst.tile([HD, HD], BF16, name="perm_t")
            ident_t = cst.tile([HD, HD], BF16, name="ident_t")
            tri_t = cst.tile([128, 128], BF16, name="tri_t")
            # per-head softmax-denominator partials, written by GpSimd
            # cross-partition reduces at free offsets (partition 0 only);
            # bf16 partials (0.4% rel) keep the slot small enough for a
            # 4th ptall buffer
            partial1 = p1p.tile([1, 8192], BF16, tag="p1", name="partial1")

            def table_loads():
                yield lambda: nc.sync.dma_start(perm_t[:], perm_d[:])
                yield lambda: nc.scalar.dma_start(ident_t[:], ident_d[:])
                yield lambda: nc.sync.dma_start(tri_t[:], tri_d[:])

            wq_tiles = [None] * DP

            # persistent activations, one tile per (tensor, s-chunk) so a
            # reader depends only on its own chunk's producer
            QTc = [[qtp.tile([HD, 512], BF16, tag="qtc", name=f"qt{h}_{c}")
                    for c in range(SC)] for h in range(QH)]
            KTc = [ktp.tile([HD, 512], BF16, tag="ktc", name=f"kt{c}")
                   for c in range(SC)]
            Vnc = [vnp.tile([128, 512], BF16, tag="vnc", name=f"vn{c}")
                   for c in range(SC)]

            # ---- phase 1: QKV projections + RoPE + V transpose ----
            def rope_copy(acc_ps, eng):
                raw = rawp.tile([128, 512], BF16, tag="raw", name="rope_raw")
                if eng == "act":
                    nc.scalar.activation(raw[:], acc_ps[:], AF.Copy)
                else:
                    nc.vector.tensor_copy(raw[:], acc_ps[:])
                return raw

            def rope_combine(raw, dest, cc, sn):
                rot = psB.tile([128, 512], F32, tag="tmp", name="rope_rot")
                nc.tensor.matmul(rot[:], perm_t[:], raw[:], start=True,
                                 stop=True)
                t1 = stg.tile([128, 512], BF16, tag="stg", name="rope_t1")
                nc.vector.tensor_mul(t1[:], raw[:], cc[:])
                t2 = stg.tile([128, 512], BF16, tag="stg", name="rope_t2")
                nc.vector.tensor_mul(t2[:], rot[:], sn[:])
                nc.vector.tensor_add(dest[:], t1[:], t2[:])

            boundary_pe = [None]

            def emit_boundary_pe():
                if boundary_pe[0] is not None:
                    boundary_pe[0]()
                    boundary_pe[0] = None

            cs_tiles = [None] * SC  # (cos, sin) chunk tiles, single-use

            # ---- phase 2: attention, interleaved with QKV by chunk ----
            otn_all = [[None] * QH for _ in range(SC)]

            def attn_chunk(c):
                njt = 4 * c + 4
                for h in range(QH):
                    qch = QTc[h][c]
                    ptall = ptp.tile([128, 512 * njt], BF16, tag="pt",
                                     name="ptall")
                    ot = psA.tile([128, 512], F32, tag="acc", name="ot_ps")

                    def score(jt, c=c, qch=qch, ptall=ptall):
                        m = jt - 4 * c
                        ls = 128 * m if m > 0 else 0
                        stp = psA.tile([128, 512], F32, tag="acc",
                                       name="stp")
                        nc.tensor.matmul(
                            stp[:, ls:512],
                            KTc[jt // 4][:, 128 * (jt % 4):128 * (jt % 4 + 1)],
                            qch[:, ls:512], start=True, stop=True)
                        base = 512 * jt
                        nc.scalar.activation(ptall[:, base + ls:base + 512],
                                             stp[:, ls:512], AF.Exp)
                        if m >= 1:
                            nc.vector.memset(ptall[:, base:base + ls], 0)
                        if m >= 0:
                            blk = slice(base + 128 * m, base + 128 * m + 128)
                            nc.vector.tensor_mul(ptall[:, blk], ptall[:, blk],
                                                 tri_t[:])

                    def accum(jt, ot=ot, ptall=ptall, njt=njt):
                        nc.tensor.matmul(
                            ot[:],
                            Vnc[jt // 4][:, 128 * (jt % 4):128 * (jt % 4 + 1)],
                            ptall[:, 512 * jt:512 * (jt + 1)],
                            start=(jt == 0), stop=(jt == njt - 1))

                    def reduce_group(g, ptall=ptall):
                        with nc.allow_low_precision(reason="bf16 denom"):
                            nc.gpsimd.tensor_reduce(
                                partial1[0:1, 2048 * g:2048 * (g + 1)],
                                ptall[:, 2048 * g:2048 * (g + 1)],
                                axis=mybir.AxisListType.C,
                                op=mybir.AluOpType.add)

                    for jt in range(njt):
                        score(jt)
                        if jt % 4 == 3:
                            reduce_group(jt // 4)
                        if jt >= 3:
                            accum(jt - 3)
                    for k in (3, 2, 1):
                        accum(njt - k)

                    # softmax tail: no TensorE instructions at all
                    dstack = denp.tile([16, 512], BF16, tag="dstack",
                                       name="dstack")
                    nc.sync.dma_start(dstack[0:njt, :],
                                      partial1[0:1, 0:512 * njt])
                    dsum = denp.tile([1, 512], F32, tag="dsum", name="dsum")
                    nc.gpsimd.tensor_reduce(dsum[:], dstack[0:njt, :],
                                            axis=mybir.AxisListType.C,
                                            op=mybir.AluOpType.add)
                    rc = denp.tile([1, 512], BF16, tag="rc", name="rc")
                    with nc.allow_low_precision(reason="bf16 softmax recip"):
                        nc.vector.reciprocal(rc[:], dsum[:])
                    rcb = denp.tile([128, 512], BF16, tag="rcb", name="rcb")
                    nc.gpsimd.partition_broadcast(rcb[:], rc[:], channels=128)
                    # drain PSUM early so the accumulator bank frees fast
                    otu = otup.tile([128, 512], BF16, tag="otu", name="otu")
                    nc.scalar.activation(otu[:], ot[:], AF.Copy)
                    otn = otnp.tile([128, 512], BF16, tag="otn", name="otn")
                    otn_all[c][h] = otn
                    nc.vector.tensor_mul(otn[:], otu[:], rcb[:])

            emit_attn = [attn_chunk]

            for sc in range(SC):
                qps = [psA.tile([128, 512], F32, tag="acc", name=f"qps{h}")
                       for h in range(QH)]
                kps = psA.tile([128, 512], F32, tag="acc", name="kps")
                vps = psA.tile([128, 512], F32, tag="acc", name="vps")
                for dp in range(DP):
                    if sc == 0:
                        wt = wpool.tile([128, 1024], BF16, tag="w",
                                        name=f"wq{dp}")
                        nc.scalar.dma_start(wt[:], wq_d[dp])
                        wq_tiles[dp] = wt
                    xt = xpool.tile([128, 1536], BF16, tag="x", name="xt")
                    if sc == 0 and dp == 0:
                        # split the very first transfer: the k/v slice lands
                        # ~1.5us sooner and the k/v matmuls don't need wq
                        nc.sync.dma_start(xt[:, 0:768], xw_d[0, 0, :, 0:768])
                        nc.scalar.dma_start(xt[:, 768:1536],
                                            xw_d[0, 0, :, 768:1536])
                    else:
                        nc.sync.dma_start(xt[:], xw_d[sc, dp])
                    if sc == 0:
                        if dp == 0:
                            _tl = table_loads()
                        next(_tl, lambda: None)()
                    if dp == 1:
                        emit_boundary_pe()
                    if dp == 8:
                        # prefetch this boundary's cos/sin chunk
                        cc = tabp.tile([128, 512], BF16, tag="tab", name="cc")
                        nc.sync.dma_start(cc[:], cos_d[sc])
                        sn = tabp.tile([128, 512], BF16, tag="tab", name="sn")
                        nc.sync.dma_start(sn[:], sin_d[sc])
                        cs_tiles[sc] = (cc, sn)
                    for half in range(2):
                        xs = xt[:, 768 * half:768 * half + 512]
                        ks = xt[:, 768 * half + 512:768 * half + 640]
                        vs = xt[:, 768 * half + 640:768 * half + 768]
                        st = dp == 0 and half == 0
                        sp = dp == DP - 1 and half == 1
                        wqh = wq_tiles[dp][:, 512 * half:512 * (half + 1)]
                        # k/v first: they only need the xw tile (not wq) at
                        # warmup, and their accumulators stop ~1us before the
                        # q heads at chunk boundaries so the drain overlaps
                        # the remaining q matmuls
                        nc.tensor.matmul(kps[:], ks, xs, start=st, stop=sp)
                        nc.tensor.matmul(vps[:], vs, xs, start=st, stop=sp)
                        for h in range(QH):
                            nc.tensor.matmul(
                                qps[h][:], wqh[:, HD * h:HD * (h + 1)],
                                xs, start=st, stop=sp)

                raw_k = rope_copy(kps, "act")
                vt_sb = rope_copy(vps, "dve")
                raw_q = [None] * QH
                raw_q[0] = rope_copy(qps[0], "dve")

                def boundary(sc=sc, raw_k=raw_k, vt_sb=vt_sb, raw_q=raw_q):
                    cc, sn = cs_tiles[sc]
                    rope_combine(raw_k, KTc[sc], cc, sn)
                    rope_combine(raw_q[0], QTc[0][sc], cc, sn)
                    vtp = psB.tile([128, 512], BF16, tag="tmp", name="vtp")
                    for k4 in range(4):
                        nc.tensor.transpose(
                            vtp[:, 128 * k4:128 * (k4 + 1)],
                            vt_sb[:, 128 * k4:128 * (k4 + 1)], ident_t[:])
                    nc.scalar.activation(Vnc[sc][:], vtp[:], AF.Copy)
                    for h in range(1, QH):
                        rope_combine(raw_q[h], QTc[h][sc], cc, sn)

                boundary_pe[0] = boundary
                if sc >= 1:
                    emit_attn[0](sc - 1)
                for h in range(1, QH):
                    raw_q[h] = rope_copy(qps[h],
                                         "act" if h % 2 == 0 else "dve")
                if sc == SC - 1:
                    # prefetch all Wo tiles now: the 16 wq slots free
                    # progressively through the last QKV chunk
                    wo_tiles = []
                    for dc in range(8):
                        woa = wpool.tile([128, 1024], BF16, tag="w",
                                         name=f"woa{dc}")
                        nc.sync.dma_start(woa[:], wo_d[dc, 0])
                        wob = wpool.tile([128, 1024], BF16, tag="w",
                                         name=f"wob{dc}")
                        nc.sync.dma_start(wob[:], wo_d[dc, 1])
                        wo_tiles.append((woa, wob))

                    emit_boundary_pe()

            # ---- phase 3: output projection (row-parallel partial) ----
            # chunk-major; attn3 is emitted BETWEEN o_proj chunks 1 and 2 so
            # the GpSimd reduce backlog from attn2 drains under o_proj c0/c1
            # instead of stalling attn3's P-buffer rotation
            def o_proj_chunk(c):
                    for dc in range(8):
                        woa, wob = wo_tiles[dc]
                        wsl = [woa[:, 0:512], woa[:, 512:1024],
                               wob[:, 0:512], wob[:, 512:1024]]
                        for lp in range(2):
                            ost = ostp.tile([128, 1024], BF16, tag="ost",
                                            name="ost")
                            for k2 in range(2):
                                kk = 2 * lp + k2
                                acc = psA.tile([128, 512], F32, tag="acc",
                                               name="oacc")
                                for hh in range(QH):
                                    nc.tensor.matmul(
                                        acc[:],
                                        otn_all[c][hh][:, 128 * kk:
                                                       128 * (kk + 1)],
                                        wsl[hh], start=(hh == 0),
                                        stop=(hh == QH - 1))
                                dstc = ost[:, 512 * k2:512 * (k2 + 1)]
                                if k2 == 0:
                                    nc.vector.tensor_copy(dstc, acc[:])
                                else:
                                    nc.scalar.activation(dstc, acc[:],
                                                         AF.Copy)
                            eng = nc.sync if lp == 0 else nc.scalar
                            eng.dma_start(out_d[dc, 2 * c + lp], ost[:])

            o_proj_chunk(0)
            o_proj_chunk(1)
            emit_attn[0](SC - 1)
            o_proj_chunk(2)
            o_proj_chunk(3)

    nc.compile()
    return nc


def _host_tables():
    pos = np.arange(S, dtype=np.float64)
    inv_freq = ROPE_BASE ** (-np.arange(0, HD, 2, dtype=np.float64) / HD)
    ang = np.outer(pos, inv_freq)  # [S, HD/2]
    cos = np.cos(ang).T.astype(np.float32)  # [HD/2, S]
    sin = np.sin(ang).T.astype(np.float32)
    cos2 = np.repeat(cos, 2, axis=0)  # [HD, S]
    sin2 = np.repeat(sin, 2, axis=0)
    sin2[0::2, :] *= -1.0  # even rows get -sin, odd rows +sin

    perm = np.zeros((HD, HD), dtype=np.float32)
    for i in range(HD):
        perm[i ^ 1, i] = 1.0

    # keep where jr <= ic: triangular boundary block shared by every
    # diagonal tile (the dead columns are zeroed by DVE memsets instead)
    jr = np.arange(128)[:, None]
    ic = np.arange(128)[None, :]
    tri = np.where(jr <= ic, 1.0, 0.0).astype(np.float32)

    return cos2, sin2, perm, tri


def kernel(x, Wq, Wk, Wv, Wo):
    global LAST_EXEC_NS
    if "nc" not in _CACHE:
        _CACHE["nc"] = _build_nc()
    nc = _CACHE["nc"]

    x = np.asarray(x, dtype=np.float32).reshape(S, D)
    Wq = np.asarray(Wq, dtype=np.float32)
    Wk = np.asarray(Wk, dtype=np.float32)
    Wv = np.asarray(Wv, dtype=np.float32)
    Wo = np.asarray(Wo, dtype=np.float32)

    xT = np.ascontiguousarray(x.T)  # [D, S]
    xTr = xT.reshape(DP * 2, 128, S).astype(BF)
    cos2, sin2, perm, tri = _host_tables()
    cos2c = np.ascontiguousarray(
        cos2.reshape(HD, SC, 512).transpose(1, 0, 2)).astype(BF)
    sin2c = np.ascontiguousarray(
        sin2.reshape(HD, SC, 512).transpose(1, 0, 2)).astype(BF)
    scale = np.float32(1.0 / np.sqrt(HD))
    ident = np.eye(HD, dtype=np.float32).astype(BF)
    perm = perm.astype(BF)
    tri = tri.astype(BF)

    in_maps = []
    for c in range(NCORES):
        qs = slice(QH * HD * c, QH * HD * (c + 1))
        ks = slice(HD * c, HD * (c + 1))
        wkv = np.concatenate([Wk[:, ks], Wv[:, ks]], axis=1)  # [D, 256]
        wkvr = wkv.reshape(DP * 2, 128, 256).astype(BF)
        # xw[sc, dp, p, :] = [x_A | wkv_A | x_B | wkv_B] for d-tile pair
        xw = np.empty((SC, DP, 128, 1536), dtype=BF)
        for sc in range(SC):
            cs = slice(512 * sc, 512 * (sc + 1))
            xw[sc, :, :, 0:512] = xTr[0::2, :, cs]
            xw[sc, :, :, 512:768] = wkvr[0::2]
            xw[sc, :, :, 768:1280] = xTr[1::2, :, cs]
            xw[sc, :, :, 1280:1536] = wkvr[1::2]

        wqc = (Wq[:, qs] * scale).reshape(DP * 2, 128, 512)
        wq2 = np.concatenate([wqc[0::2], wqc[1::2]], axis=2).astype(BF)

        # wo4[dc, half, p, 512*(hh%2)+col] = Wo[128*(2*half+hh%2)+p,
        #                                        512*dc+col]
        wo4 = np.ascontiguousarray(
            Wo[qs, :].reshape(2, 2, 128, 8, 512).transpose(3, 0, 2, 1, 4)
            .reshape(8, 2, 128, 1024)).astype(BF)

        in_maps.append({
            "xw": xw,
            "wq2": np.ascontiguousarray(wq2),
            "wo4": wo4,
            "cos2": cos2c,
            "sin2": sin2c,
            "perm": perm,
            "tri": tri,
            "ident": ident,
        })

    res = run_bass_kernel_spmd(nc, in_maps, list(range(NCORES)),
                               trace=TRACE)
    LAST_EXEC_NS = res.exec_time_ns

    acc = res.results[0]["out"].astype(np.float32)
    for c in range(1, NCORES):
        acc = acc + res.results[c]["out"].astype(np.float32)
    # out[dc, sp2, p, k2*512 + col] -> out[(2*sp2+k2)*128 + p, dc*512 + col]
    out = (acc.reshape(8, 8, 128, 2, 512).transpose(1, 3, 2, 0, 4)
           .reshape(S, D))
    return np.ascontiguousarray(out).reshape(1, S, D)
